# revision 31
# baseline (speedup 1.0000x reference)
"""Trainium2 Bass kernel for a single attention head (B=4, S=2048, D=4096, DH=128).

Sharding: 8 cores = (batch b, half h), pair (2b, 2b+1) shares batch b.
Core (b,h) owns the INTERLEAVED tile set {t : t % 2 == h} (8 tiles of 128
positions) -- both its q rows AND its K/V key chunks. Each core projects
Q, K, V only for its OWN 1024 columns of x (half the work of replicating
K/V), then the pair exchanges K/V halves with pairwise AllGathers.

Rank-symmetric layout (same NEFF on every core): K^T / V go through a
DRAM bounce + AllGather and are read back RANK-indexed (rank0 = even
tiles, rank1 = odd tiles), so no access pattern depends on h. The causal
structure relative to rank0/rank1 is baked into the host-provided mask
block (triangle / zeros / -1e9).

Phases:
  1: two column-stages (own cols 0:512, 512:1024), K/V projections only
     (Q deferred). Gather 1 (K stage A) fires as early as possible so
     small tiles' logits can interleave with the deferred Q matmuls;
     gather 2 (K stage B + V both stages) fires after stage B drains and
     its latency hides under Q + small-tile work.
  2: per local q-tile j (global 2j+h): logits over rank0 range
     [0:(j+1)*128] and rank1 range [1024:1024+(j+1)*128], mask on the
     last chunk of each range; per-group rowmax (bf16 rounding is
     monotone so group-max == full max) -> exp (accum rowsum) -> W^T via
     one DMA transpose -> PV -> scale -> out. ALL logits precede ALL pv
     stages (the PE queue is in-order; a stalled pv would head-of-line
     block independent logits). Output DMAs ride gpsimd, transposes
     sync, exp scalar, everything else vector.
"""

import numpy as np
import ml_dtypes

import concourse.bass as bass
import concourse.tile as tile
from concourse import bacc, mybir
from concourse.bass_utils import run_bass_kernel_spmd

B, S, D, DH = 4, 2048, 4096, 128
SQ = S // 2          # own q rows / own key cols per core
ST = 512             # columns per stage
N_CORES = 8
D_CH = D // 128      # 32 contraction chunks
NT = 8               # local q tiles (slots)
PAIRS = [[0, 1], [2, 3], [4, 5], [6, 7]]

BF16 = mybir.dt.bfloat16
F32 = mybir.dt.float32


def build_nc():
    nc = bacc.Bacc(None)

    # x own columns, stage-major: xT[st*128+p, i, s] = x[b, col(st*512+s), i*128+p]
    xT = nc.dram_tensor("xT", [2 * 128, D_CH, ST], BF16, kind="ExternalInput")
    mask = nc.dram_tensor("mask", [128, 256], BF16, kind="ExternalInput")
    # weights pre-tiled on host: w[p, i, m] = W[m, i*128+p]
    wqT = nc.dram_tensor("wqT", [128, D_CH, DH], BF16, kind="ExternalInput")
    wkT = nc.dram_tensor("wkT", [128, D_CH, DH], BF16, kind="ExternalInput")
    wvT = nc.dram_tensor("wvT", [128, D_CH, DH], BF16, kind="ExternalInput")
    bq = nc.dram_tensor("bq", [DH, 1], F32, kind="ExternalInput")
    bk = nc.dram_tensor("bk", [DH, 1], F32, kind="ExternalInput")
    bv = nc.dram_tensor("bv", [DH, 1], F32, kind="ExternalInput")
    out = nc.dram_tensor("out", [SQ, DH], BF16, kind="ExternalOutput")
    # comm bounces: gather 1 = K^T stage A (early -- unlocks small tiles'
    # logits during the deferred-Q window); gather 2 = K^T stage B + V both
    cck_in = nc.dram_tensor("cck_in", [128, ST], BF16, kind="Internal")
    cck_out = nc.dram_tensor("cck_out", [256, ST], BF16, kind="Internal")
    cc2_in = nc.dram_tensor("cc2_in", [128, 3 * ST], BF16, kind="Internal")
    cc2_out = nc.dram_tensor("cc2_out", [256, 3 * ST], BF16, kind="Internal")
    # tiny warmup collective: absorbs first-call TOPSP/communicator latency
    ccw_in = nc.dram_tensor("ccw_in", [1, 64], BF16, kind="Internal")
    ccw_out = nc.dram_tensor("ccw_out", [2, 64], BF16, kind="Internal")

    with tile.TileContext(nc) as tc:
        with (
            tc.tile_pool(name="weights", bufs=1) as wpool,
            tc.tile_pool(name="persist", bufs=1) as persist,
        ):
            w_sb = {}
            for name in ("q", "k", "v"):
                w_sb[name] = wpool.tile([128, D_CH, DH], BF16, tag=f"w{name}",
                                        name=f"w{name}")
            W_EXT = {"q": wqT, "k": wkT, "v": wvT}
            b_sb = {}
            for name in ("q", "k", "v"):
                t = wpool.tile([DH, 1], F32, tag=f"b{name}", name=f"b{name}")
                b_sb[name] = t
            nc.gpsimd.dma_start(out=ccw_in.ap(), in_=mask[0:1, 0:64])
            nc.gpsimd.collective_compute(
                "AllGather", mybir.AluOpType.bypass,
                replica_groups=PAIRS,
                ins=[ccw_in.ap().opt()], outs=[ccw_out.ap().opt()],
            )
            mk = persist.tile([128, 256], BF16, tag="mk")
            nc.gpsimd.dma_start(out=mk[:], in_=mask[:])
            for name, ext in (("q", bq), ("k", bk), ("v", bv)):
                nc.gpsimd.dma_start(out=b_sb[name][:], in_=ext[:])

            # rank-indexed: rank0 keys at [0:1024], rank1 at [1024:2048]
            kt_sb = persist.tile([DH, S], BF16, tag="kt")
            v_sb = persist.tile([128, 2 * NT, DH], BF16, tag="v")
            qt_sb = persist.tile([DH, SQ], BF16, tag="qt")   # own Q^T
            x_st = [persist.tile([128, D_CH, ST], BF16, tag=f"x{st}",
                                 name=f"x{st}") for st in (0, 1)]
            kst = [persist.tile([DH, ST], BF16, tag=f"kst{st}", name=f"kst{st}")
                   for st in (0, 1)]
            vst = [persist.tile([DH, ST], BF16, tag=f"vst{st}", name=f"vst{st}")
                   for st in (0, 1)]
            vtr = [persist.tile([128, 4, DH], BF16, tag=f"vtr{st}", name=f"vtr{st}")
                   for st in (0, 1)]

            # Q accumulators persist into phase 2 (small-tile logits
            # interleave between Q_A and Q_B): own 2 PSUM banks for the
            # whole kernel; K/V accumulators get a pool closed before
            # phase 2 opens its logits pool. 2 + 4 (kv) <= 8; later
            # 2 + 4 (lg) + 2 (o) = 8.
            ppq_cm = tc.tile_pool(name="ppsumq", bufs=1, space="PSUM")
            ppq = ppq_cm.__enter__()
            acc = {}
            for tag in ("pq0", "pq1"):
                acc[tag] = ppq.tile([DH, ST], F32, tag=tag, name=tag)

            # --- phase 1: K/V projections in two column-stages + gathers ---
            with tc.tile_pool(name="ppsumkv", bufs=1, space="PSUM") as ppkv:
                for tag in ("pk0", "pv0", "pk1", "pv1"):
                    acc[tag] = ppkv.tile([DH, ST], F32, tag=tag, name=tag)

                def x_piece(st, i0, n):
                    nc.sync.dma_start(out=x_st[st][:, i0:i0 + n, :],
                                      in_=xT[st * 128:(st + 1) * 128, i0:i0 + n, :])

                def wsl(name, i0, n):
                    ss = np.s_[:, i0:i0 + n, :]
                    nc.sync.dma_start(out=w_sb[name][ss], in_=W_EXT[name][ss])

                # single-queue x/w stream in consumption order (k/v weights
                # early, q weights before stage-B x since Q is deferred);
                # nothing else rides sync in phase 1, so the stream is
                # never blocked behind a wait
                x_piece(0, 0, 2)
                wsl("k", 0, 4)
                wsl("v", 0, 4)
                x_piece(0, 2, 2)
                x_piece(0, 4, 4)
                wsl("k", 4, 12)
                wsl("v", 4, 12)
                x_piece(0, 8, 8)
                x_piece(0, 16, 8)
                wsl("k", 16, 16)
                wsl("v", 16, 16)
                x_piece(0, 24, 8)
                wsl("q", 0, 16)
                wsl("q", 16, 16)
                for i0 in (0, 8, 16, 24):
                    x_piece(1, i0, 8)

                def stage_mms(st):
                    for i in range(D_CH):
                        stt = dict(start=(i == 0), stop=(i == D_CH - 1))
                        nc.tensor.matmul(acc[f"pk{st}"][:], lhsT=w_sb["k"][:, i, :],
                                         rhs=x_st[st][:, i, :], **stt)
                        nc.tensor.matmul(acc[f"pv{st}"][:], lhsT=w_sb["v"][:, i, :],
                                         rhs=x_st[st][:, i, :], **stt)

                def stage_drain(st):
                    nc.vector.tensor_scalar_add(kst[st][:], acc[f"pk{st}"][:],
                                                b_sb["k"][:])
                    nc.vector.tensor_scalar_add(vst[st][:], acc[f"pv{st}"][:],
                                                b_sb["v"][:])
                    nc.scalar.dma_start_transpose(out=vtr[st][:], in_=vst[st][:])

                stage_mms(0)
                stage_drain(0)
                stage_mms(1)
                stage_drain(1)
                # gather 1: K stage A
                nc.gpsimd.dma_start(out=cck_in.ap(), in_=kst[0][:])
                nc.gpsimd.collective_compute(
                    "AllGather", mybir.AluOpType.bypass,
                    replica_groups=PAIRS,
                    ins=[cck_in.ap().opt()], outs=[cck_out.ap().opt()],
                )
                nc.gpsimd.dma_start(out=kt_sb[:, 0:ST], in_=cck_out[0:128, :])
                nc.gpsimd.dma_start(out=kt_sb[:, SQ:SQ + ST],
                                    in_=cck_out[128:256, :])
                # gather 2: K stage B + V both stages
                nc.gpsimd.dma_start(out=cc2_in[:, 0:ST], in_=kst[1][:])
                nc.gpsimd.dma_start(out=cc2_in[:, ST:2 * ST], in_=vtr[0][:])
                nc.gpsimd.dma_start(out=cc2_in[:, 2 * ST:3 * ST], in_=vtr[1][:])
                nc.gpsimd.collective_compute(
                    "AllGather", mybir.AluOpType.bypass,
                    replica_groups=PAIRS,
                    ins=[cc2_in.ap().opt()], outs=[cc2_out.ap().opt()],
                )
                nc.gpsimd.dma_start(out=kt_sb[:, ST:SQ], in_=cc2_out[0:128, 0:ST])
                nc.gpsimd.dma_start(out=kt_sb[:, SQ + ST:S],
                                    in_=cc2_out[128:256, 0:ST])
                nc.gpsimd.dma_start(out=v_sb[:, 0:NT, :],
                                    in_=cc2_out[0:128, ST:3 * ST])
                nc.gpsimd.dma_start(out=v_sb[:, NT:2 * NT, :],
                                    in_=cc2_out[128:256, ST:3 * ST])

            # --- phase 2 (deferred Q interleaved) ---
            with (
                tc.tile_pool(name="lg_psum", bufs=4, space="PSUM") as lg,
                tc.tile_pool(name="o_psum", bufs=2, space="PSUM") as opool,
                tc.tile_pool(name="lm_sb", bufs=1) as lmpool,
                tc.tile_pool(name="wt_sb", bufs=1) as wtpool,
                tc.tile_pool(name="stats", bufs=12) as stat,
                tc.tile_pool(name="out_sb", bufs=2) as ospool,
            ):
                pv_args = {}

                def q_proj(st):
                    for i in range(D_CH):
                        nc.tensor.matmul(acc[f"pq{st}"][:], lhsT=w_sb["q"][:, i, :],
                                         rhs=x_st[st][:, i, :],
                                         start=(i == 0), stop=(i == D_CH - 1))
                    nc.vector.tensor_scalar_add(qt_sb[:, st * ST:(st + 1) * ST],
                                                acc[f"pq{st}"][:], b_sb["q"][:])

                def softmax_stage(j):
                    e = j + 1            # chunks per rank range
                    w = e * 128          # cols per rank range
                    qsl = np.s_[:, j * 128:(j + 1) * 128]

                    # Only the LAST chunk of each rank range carries mask
                    # values (host bakes h into mk): rank0 last = triangle
                    # (h=0) or all-0 (h=1); rank1 last = all--1e9 (h=0) or
                    # triangle (h=1). Other chunks are pure past: plain
                    # psum f32 -> bf16 rounding. Rowmax is per-group as
                    # each group's bf16 logits land (rounding is monotone,
                    # so group-wise max == full max) + one tiny combine.
                    lmt = lmpool.tile([128, 2 * w], BF16, tag=f"lm{j}")
                    gms = stat.tile([128, 8], F32, tag="gms")
                    ng = 0
                    for base, off, mcol in ((0, 0, 0), (SQ, w, 128)):
                        for g0 in range(0, w, 512):
                            gw = min(512, w - g0)
                            pg = lg.tile([128, 512], F32, tag="pg")
                            nc.tensor.matmul(pg[:, :gw], lhsT=qt_sb[qsl],
                                             rhs=kt_sb[:, base + g0:base + g0 + gw],
                                             start=True, stop=True)
                            last = g0 + gw == w
                            cp = gw - 128 if last else gw
                            if cp:
                                nc.vector.tensor_copy(
                                    lmt[:, off + g0:off + g0 + cp], pg[:, :cp])
                            if last:
                                nc.vector.tensor_add(
                                    lmt[:, off + g0 + cp:off + g0 + gw],
                                    pg[:, cp:gw],
                                    mk[:, mcol:mcol + 128])
                            nc.vector.reduce_max(
                                out=gms[:, ng:ng + 1],
                                in_=lmt[:, off + g0:off + g0 + gw],
                                axis=mybir.AxisListType.X)
                            ng += 1
                    negmax = stat.tile([128, 1], F32, tag="negmax")
                    nc.vector.reduce_max(out=negmax[:], in_=gms[:, :ng],
                                         axis=mybir.AxisListType.X, negate=True)
                    w_t = lmpool.tile([128, 2 * w], BF16, tag=f"w{j}")
                    wt_t = wtpool.tile([128, 2 * e, 128], BF16, tag=f"wt{j}")
                    sumexp = stat.tile([128, 1], F32, tag="sumexp")
                    nc.scalar.activation(
                        out=w_t[:, :2 * w], in_=lmt[:, :2 * w],
                        func=mybir.ActivationFunctionType.Exp,
                        bias=negmax[:], scale=1.0, accum_out=sumexp[:])
                    nc.sync.dma_start_transpose(out=wt_t[:, :2 * e, :],
                                                in_=w_t[:, :2 * w])
                    pv_args[j] = (wt_t, sumexp, e)

                def pv_stage(j):
                    wt_t, sumexp, e = pv_args.pop(j)
                    rsum = stat.tile([128, 1], F32, tag="rsum")
                    nc.vector.reciprocal(rsum[:], sumexp[:])
                    po = opool.tile([128, DH], F32, tag="po")
                    for c in range(2 * e):
                        vc = c if c < e else NT + (c - e)
                        nc.tensor.matmul(po[:], lhsT=wt_t[:, c, :], rhs=v_sb[:, vc, :],
                                         start=(c == 0), stop=(c == 2 * e - 1))
                    o_sb = ospool.tile([128, DH], BF16, tag="o")
                    nc.vector.tensor_scalar_mul(o_sb[:], po[:], rsum[:])
                    nc.gpsimd.dma_start(out=out[j * 128:(j + 1) * 128, :],
                                        in_=o_sb[:])

                # PE program order: Q_A -> small-tile logits (gated only on
                # gather 1) -> Q_B -> big-tile logits (gather 2 arrives
                # under Q/smalls) -> ALL pv stages (in chain-completion
                # order). ALL logits precede ALL pv: the PE queue is
                # in-order and a stalled pv would block independent logits.
                q_proj(0)
                softmax_stage(3)
                softmax_stage(2)
                softmax_stage(1)
                softmax_stage(0)
                q_proj(1)
                softmax_stage(7)
                softmax_stage(6)
                softmax_stage(5)
                softmax_stage(4)
                pv_stage(3)
                pv_stage(2)
                pv_stage(1)
                pv_stage(0)
                pv_stage(7)
                pv_stage(6)
                pv_stage(5)
                pv_stage(4)
            ppq_cm.__exit__(None, None, None)

    nc.finalize()
    return nc


def shard_inputs(x, attn_mask, Wq, bq, Wk, bk, Wv, bv):
    """Host-side shard prep. Returns in_maps for cores 0..7."""
    bf = ml_dtypes.bfloat16
    xb = np.asarray(x).astype(bf)                   # cast first, like the reference
    mask_f = np.asarray(attn_mask)

    def tile_w(W):
        WT = np.asarray(W).astype(bf).T.reshape(D_CH, 128, DH)
        return np.ascontiguousarray(WT.transpose(1, 0, 2))

    wqt, wkt, wvt = tile_w(Wq), tile_w(Wk), tile_w(Wv)
    bqc = np.asarray(bq).astype(bf).astype(np.float32).reshape(DH, 1)
    bkc = np.asarray(bk).astype(bf).astype(np.float32).reshape(DH, 1)
    bvc = np.asarray(bv).astype(bf).astype(np.float32).reshape(DH, 1)

    tri = mask_f[:128, :128].astype(bf)       # causal triangle (0/-1e9)
    zeros = np.zeros((128, 128), dtype=bf)
    neg = np.full((128, 128), -1e9, dtype=np.float32).astype(bf)

    in_maps = []
    for c in range(N_CORES):
        b, h = divmod(c, 2)
        own = np.concatenate([np.arange(t * 128, (t + 1) * 128)
                              for t in range(h, 16, 2)])
        xcols = xb[b][own]                            # [1024, D]
        # [st, s, i, p] -> [st, p, i, s], flatten stage into rows
        xa = np.ascontiguousarray(
            xcols.reshape(2, ST, D_CH, 128).transpose(0, 3, 2, 1)
        ).reshape(2 * 128, D_CH, ST)
        if h == 0:
            msk = np.concatenate([tri, neg], axis=1)       # rank0 diag, rank1 future
        else:
            msk = np.concatenate([zeros, tri], axis=1)     # rank0 past, rank1 diag
        in_maps.append({
            "xT": xa, "mask": np.ascontiguousarray(msk),
            "wqT": wqt, "wkT": wkt, "wvT": wvt,
            "bq": bqc, "bk": bkc, "bv": bvc,
        })
    return in_maps


_NC_CACHE = {}


def kernel(x, attn_mask, Wq, bq, Wk, bk, Wv, bv):
    if "nc" not in _NC_CACHE:
        _NC_CACHE["nc"] = build_nc()
    nc = _NC_CACHE["nc"]
    in_maps = shard_inputs(x, attn_mask, Wq, bq, Wk, bk, Wv, bv)
    res = run_bass_kernel_spmd(nc, in_maps, list(range(N_CORES)))
    out = np.empty((B, S, DH), dtype=ml_dtypes.bfloat16)
    for c in range(N_CORES):
        b, h = divmod(c, 2)
        for j in range(NT):
            t = 2 * j + h
            out[b, t * 128:(t + 1) * 128, :] = res.results[c]["out"][j * 128:(j + 1) * 128]
    return out


# revision 32
# speedup vs baseline: 1.0756x; 1.0756x over previous
"""Trainium2 Bass kernel for a single attention head (B=4, S=2048, D=4096, DH=128).

Sharding: 8 cores = (batch b, half h), pair (2b, 2b+1) shares batch b.
Core (b,h) owns the INTERLEAVED tile set {t : t % 2 == h} (8 tiles of 128
positions) -- both its q rows AND its K/V key chunks. Each core projects
Q, K, V only for its OWN 1024 columns of x (half the work of replicating
K/V), then the pair exchanges K/V halves with pairwise AllGathers.

Rank-symmetric layout (same NEFF on every core): K^T / V go through a
DRAM bounce + AllGather and are read back RANK-indexed (rank0 = even
tiles, rank1 = odd tiles), so no access pattern depends on h. The causal
structure relative to rank0/rank1 is baked into the host-provided mask
block (triangle / zeros / -1e9).

Phases:
  1: two column-stages (own cols 0:512, 512:1024), K/V projections only
     (Q deferred). Gather 1 (K stage A) fires as early as possible so
     small tiles' logits can interleave with the deferred Q matmuls;
     gather 2 (K stage B + V both stages) fires after stage B drains and
     its latency hides under Q + small-tile work.
  2: per local q-tile j (global 2j+h): logits over rank0 range
     [0:(j+1)*128] and rank1 range [1024:1024+(j+1)*128], mask on the
     last chunk of each range; per-group rowmax (bf16 rounding is
     monotone so group-max == full max) -> exp (accum rowsum) -> W^T via
     one DMA transpose -> PV -> scale -> out. ALL logits precede ALL pv
     stages (the PE queue is in-order; a stalled pv would head-of-line
     block independent logits). Output DMAs ride gpsimd, transposes
     sync, exp scalar, everything else vector.
"""

import numpy as np
import ml_dtypes

import concourse.bass as bass
import concourse.tile as tile
from concourse import bacc, mybir
from concourse.bass_utils import run_bass_kernel_spmd

B, S, D, DH = 4, 2048, 4096, 128
SQ = S // 2          # own q rows / own key cols per core
ST = 512             # columns per stage
N_CORES = 8
D_CH = D // 128      # 32 contraction chunks
NT = 8               # local q tiles (slots)
PAIRS = [[0, 1], [2, 3], [4, 5], [6, 7]]

BF16 = mybir.dt.bfloat16
F32 = mybir.dt.float32


def build_nc():
    nc = bacc.Bacc(None)

    # x own columns, stage-major: xT[st*128+p, i, s] = x[b, col(st*512+s), i*128+p]
    xT = nc.dram_tensor("xT", [2 * 128, D_CH, ST], BF16, kind="ExternalInput")
    mask = nc.dram_tensor("mask", [128, 256], BF16, kind="ExternalInput")
    # weights pre-tiled on host: w[p, i, m] = W[m, i*128+p]
    wqT = nc.dram_tensor("wqT", [128, D_CH, DH], BF16, kind="ExternalInput")
    wkT = nc.dram_tensor("wkT", [128, D_CH, DH], BF16, kind="ExternalInput")
    wvT = nc.dram_tensor("wvT", [128, D_CH, DH], BF16, kind="ExternalInput")
    bq = nc.dram_tensor("bq", [DH, 1], F32, kind="ExternalInput")
    bk = nc.dram_tensor("bk", [DH, 1], F32, kind="ExternalInput")
    bv = nc.dram_tensor("bv", [DH, 1], F32, kind="ExternalInput")
    out = nc.dram_tensor("out", [SQ, DH], BF16, kind="ExternalOutput")
    # comm bounces: gather 1 = K^T stage A (early -- unlocks small tiles'
    # logits during the deferred-Q window); gather 2 = K^T stage B + V both
    cck_in = nc.dram_tensor("cck_in", [128, ST], BF16, kind="Internal")
    cck_out = nc.dram_tensor("cck_out", [256, ST], BF16, kind="Internal")
    cc2_in = nc.dram_tensor("cc2_in", [128, 3 * ST], BF16, kind="Internal")
    cc2_out = nc.dram_tensor("cc2_out", [256, 3 * ST], BF16, kind="Internal")

    with tile.TileContext(nc) as tc:
        with (
            tc.tile_pool(name="weights", bufs=1) as wpool,
            tc.tile_pool(name="persist", bufs=1) as persist,
        ):
            w_sb = {}
            for name in ("q", "k", "v"):
                w_sb[name] = wpool.tile([128, D_CH, DH], BF16, tag=f"w{name}",
                                        name=f"w{name}")
            W_EXT = {"q": wqT, "k": wkT, "v": wvT}
            b_sb = {}
            for name in ("q", "k", "v"):
                t = wpool.tile([DH, 1], F32, tag=f"b{name}", name=f"b{name}")
                b_sb[name] = t
            mk = persist.tile([128, 256], BF16, tag="mk")
            nc.gpsimd.dma_start(out=mk[:], in_=mask[:])
            for name, ext in (("q", bq), ("k", bk), ("v", bv)):
                nc.gpsimd.dma_start(out=b_sb[name][:], in_=ext[:])

            # rank-indexed: rank0 keys at [0:1024], rank1 at [1024:2048]
            kt_sb = persist.tile([DH, S], BF16, tag="kt")
            v_sb = persist.tile([128, 2 * NT, DH], BF16, tag="v")
            qt_sb = persist.tile([DH, SQ], BF16, tag="qt")   # own Q^T
            x_st = [persist.tile([128, D_CH, ST], BF16, tag=f"x{st}",
                                 name=f"x{st}") for st in (0, 1)]
            kst = [persist.tile([DH, ST], BF16, tag=f"kst{st}", name=f"kst{st}")
                   for st in (0, 1)]
            vst = [persist.tile([DH, ST], BF16, tag=f"vst{st}", name=f"vst{st}")
                   for st in (0, 1)]
            vtr = [persist.tile([128, 4, DH], BF16, tag=f"vtr{st}", name=f"vtr{st}")
                   for st in (0, 1)]

            # Q accumulators persist into phase 2 (small-tile logits
            # interleave between Q_A and Q_B): own 2 PSUM banks for the
            # whole kernel; K/V accumulators get a pool closed before
            # phase 2 opens its logits pool. 2 + 4 (kv) <= 8; later
            # 2 + 4 (lg) + 2 (o) = 8.
            ppq_cm = tc.tile_pool(name="ppsumq", bufs=1, space="PSUM")
            ppq = ppq_cm.__enter__()
            acc = {}
            for tag in ("pq0", "pq1"):
                acc[tag] = ppq.tile([DH, ST], F32, tag=tag, name=tag)

            # --- phase 1: K/V projections in two column-stages + gathers ---
            with tc.tile_pool(name="ppsumkv", bufs=1, space="PSUM") as ppkv:
                for tag in ("pk0", "pv0", "pk1", "pv1"):
                    acc[tag] = ppkv.tile([DH, ST], F32, tag=tag, name=tag)

                def x_piece(st, i0, n):
                    nc.sync.dma_start(out=x_st[st][:, i0:i0 + n, :],
                                      in_=xT[st * 128:(st + 1) * 128, i0:i0 + n, :])

                def wsl(name, i0, n):
                    ss = np.s_[:, i0:i0 + n, :]
                    nc.sync.dma_start(out=w_sb[name][ss], in_=W_EXT[name][ss])

                # single-queue x/w stream in consumption order (k/v weights
                # early, q weights before stage-B x since Q is deferred);
                # nothing else rides sync in phase 1, so the stream is
                # never blocked behind a wait
                x_piece(0, 0, 2)
                wsl("k", 0, 4)
                wsl("v", 0, 4)
                x_piece(0, 2, 2)
                x_piece(0, 4, 4)
                wsl("k", 4, 12)
                wsl("v", 4, 12)
                x_piece(0, 8, 8)
                x_piece(0, 16, 8)
                wsl("k", 16, 16)
                wsl("v", 16, 16)
                x_piece(0, 24, 8)
                wsl("q", 0, 16)
                wsl("q", 16, 16)
                for i0 in (0, 8, 16, 24):
                    x_piece(1, i0, 8)

                def stage_mms(st):
                    for i in range(D_CH):
                        stt = dict(start=(i == 0), stop=(i == D_CH - 1))
                        nc.tensor.matmul(acc[f"pk{st}"][:], lhsT=w_sb["k"][:, i, :],
                                         rhs=x_st[st][:, i, :], **stt)
                        nc.tensor.matmul(acc[f"pv{st}"][:], lhsT=w_sb["v"][:, i, :],
                                         rhs=x_st[st][:, i, :], **stt)

                def stage_drain(st):
                    nc.vector.tensor_scalar_add(kst[st][:], acc[f"pk{st}"][:],
                                                b_sb["k"][:])
                    nc.vector.tensor_scalar_add(vst[st][:], acc[f"pv{st}"][:],
                                                b_sb["v"][:])
                    nc.scalar.dma_start_transpose(out=vtr[st][:], in_=vst[st][:])

                stage_mms(0)
                stage_drain(0)
                stage_mms(1)
                stage_drain(1)
                # gather 1: K stage A
                nc.gpsimd.dma_start(out=cck_in.ap(), in_=kst[0][:])
                nc.gpsimd.collective_compute(
                    "AllGather", mybir.AluOpType.bypass,
                    replica_groups=PAIRS,
                    ins=[cck_in.ap().opt()], outs=[cck_out.ap().opt()],
                )
                nc.gpsimd.dma_start(out=kt_sb[:, 0:ST], in_=cck_out[0:128, :])
                nc.gpsimd.dma_start(out=kt_sb[:, SQ:SQ + ST],
                                    in_=cck_out[128:256, :])
                # gather 2: K stage B + V both stages
                nc.gpsimd.dma_start(out=cc2_in[:, 0:ST], in_=kst[1][:])
                nc.gpsimd.dma_start(out=cc2_in[:, ST:2 * ST], in_=vtr[0][:])
                nc.gpsimd.dma_start(out=cc2_in[:, 2 * ST:3 * ST], in_=vtr[1][:])
                nc.gpsimd.collective_compute(
                    "AllGather", mybir.AluOpType.bypass,
                    replica_groups=PAIRS,
                    ins=[cc2_in.ap().opt()], outs=[cc2_out.ap().opt()],
                )
                nc.gpsimd.dma_start(out=kt_sb[:, ST:SQ], in_=cc2_out[0:128, 0:ST])
                nc.gpsimd.dma_start(out=kt_sb[:, SQ + ST:S],
                                    in_=cc2_out[128:256, 0:ST])
                nc.gpsimd.dma_start(out=v_sb[:, 0:NT, :],
                                    in_=cc2_out[0:128, ST:3 * ST])
                nc.gpsimd.dma_start(out=v_sb[:, NT:2 * NT, :],
                                    in_=cc2_out[128:256, ST:3 * ST])

            # --- phase 2 (deferred Q interleaved) ---
            with (
                tc.tile_pool(name="lg_psum", bufs=4, space="PSUM") as lg,
                tc.tile_pool(name="o_psum", bufs=2, space="PSUM") as opool,
                tc.tile_pool(name="lm_sb", bufs=1) as lmpool,
                tc.tile_pool(name="wt_sb", bufs=1) as wtpool,
                tc.tile_pool(name="stats", bufs=12) as stat,
                tc.tile_pool(name="out_sb", bufs=2) as ospool,
            ):
                pv_args = {}

                def q_proj(st):
                    for i in range(D_CH):
                        nc.tensor.matmul(acc[f"pq{st}"][:], lhsT=w_sb["q"][:, i, :],
                                         rhs=x_st[st][:, i, :],
                                         start=(i == 0), stop=(i == D_CH - 1))
                    nc.vector.tensor_scalar_add(qt_sb[:, st * ST:(st + 1) * ST],
                                                acc[f"pq{st}"][:], b_sb["q"][:])

                def softmax_stage(j):
                    e = j + 1            # chunks per rank range
                    w = e * 128          # cols per rank range
                    qsl = np.s_[:, j * 128:(j + 1) * 128]

                    # Only the LAST chunk of each rank range carries mask
                    # values (host bakes h into mk): rank0 last = triangle
                    # (h=0) or all-0 (h=1); rank1 last = all--1e9 (h=0) or
                    # triangle (h=1). Other chunks are pure past: plain
                    # psum f32 -> bf16 rounding. Rowmax is per-group as
                    # each group's bf16 logits land (rounding is monotone,
                    # so group-wise max == full max) + one tiny combine.
                    lmt = lmpool.tile([128, 2 * w], BF16, tag=f"lm{j}")
                    gms = stat.tile([128, 8], F32, tag="gms")
                    ng = 0
                    for base, off, mcol in ((0, 0, 0), (SQ, w, 128)):
                        for g0 in range(0, w, 512):
                            gw = min(512, w - g0)
                            pg = lg.tile([128, 512], F32, tag="pg")
                            nc.tensor.matmul(pg[:, :gw], lhsT=qt_sb[qsl],
                                             rhs=kt_sb[:, base + g0:base + g0 + gw],
                                             start=True, stop=True)
                            last = g0 + gw == w
                            cp = gw - 128 if last else gw
                            if cp:
                                nc.vector.tensor_copy(
                                    lmt[:, off + g0:off + g0 + cp], pg[:, :cp])
                            if last:
                                nc.vector.tensor_add(
                                    lmt[:, off + g0 + cp:off + g0 + gw],
                                    pg[:, cp:gw],
                                    mk[:, mcol:mcol + 128])
                            nc.vector.reduce_max(
                                out=gms[:, ng:ng + 1],
                                in_=lmt[:, off + g0:off + g0 + gw],
                                axis=mybir.AxisListType.X)
                            ng += 1
                    negmax = stat.tile([128, 1], F32, tag="negmax")
                    nc.vector.reduce_max(out=negmax[:], in_=gms[:, :ng],
                                         axis=mybir.AxisListType.X, negate=True)
                    w_t = lmpool.tile([128, 2 * w], BF16, tag=f"w{j}")
                    wt_t = wtpool.tile([128, 2 * e, 128], BF16, tag=f"wt{j}")
                    sumexp = stat.tile([128, 1], F32, tag="sumexp")
                    nc.scalar.activation(
                        out=w_t[:, :2 * w], in_=lmt[:, :2 * w],
                        func=mybir.ActivationFunctionType.Exp,
                        bias=negmax[:], scale=1.0, accum_out=sumexp[:])
                    nc.sync.dma_start_transpose(out=wt_t[:, :2 * e, :],
                                                in_=w_t[:, :2 * w])
                    pv_args[j] = (wt_t, sumexp, e)

                def pv_stage(j):
                    wt_t, sumexp, e = pv_args.pop(j)
                    rsum = stat.tile([128, 1], F32, tag="rsum")
                    nc.vector.reciprocal(rsum[:], sumexp[:])
                    po = opool.tile([128, DH], F32, tag="po")
                    for c in range(2 * e):
                        vc = c if c < e else NT + (c - e)
                        nc.tensor.matmul(po[:], lhsT=wt_t[:, c, :], rhs=v_sb[:, vc, :],
                                         start=(c == 0), stop=(c == 2 * e - 1))
                    o_sb = ospool.tile([128, DH], BF16, tag="o")
                    nc.vector.tensor_scalar_mul(o_sb[:], po[:], rsum[:])
                    nc.gpsimd.dma_start(out=out[j * 128:(j + 1) * 128, :],
                                        in_=o_sb[:])

                # PE program order: Q_A -> small-tile logits (gated only on
                # gather 1) -> Q_B -> big-tile logits (gather 2 arrives
                # under Q/smalls) -> ALL pv stages (in chain-completion
                # order). ALL logits precede ALL pv: the PE queue is
                # in-order and a stalled pv would block independent logits.
                q_proj(0)
                softmax_stage(3)
                softmax_stage(2)
                softmax_stage(1)
                softmax_stage(0)
                q_proj(1)
                softmax_stage(7)
                softmax_stage(6)
                softmax_stage(5)
                softmax_stage(4)
                pv_stage(3)
                pv_stage(2)
                pv_stage(1)
                pv_stage(0)
                pv_stage(7)
                pv_stage(6)
                pv_stage(5)
                pv_stage(4)
            ppq_cm.__exit__(None, None, None)

    nc.finalize()
    return nc


def shard_inputs(x, attn_mask, Wq, bq, Wk, bk, Wv, bv):
    """Host-side shard prep. Returns in_maps for cores 0..7."""
    bf = ml_dtypes.bfloat16
    xb = np.asarray(x).astype(bf)                   # cast first, like the reference
    mask_f = np.asarray(attn_mask)

    def tile_w(W):
        WT = np.asarray(W).astype(bf).T.reshape(D_CH, 128, DH)
        return np.ascontiguousarray(WT.transpose(1, 0, 2))

    wqt, wkt, wvt = tile_w(Wq), tile_w(Wk), tile_w(Wv)
    bqc = np.asarray(bq).astype(bf).astype(np.float32).reshape(DH, 1)
    bkc = np.asarray(bk).astype(bf).astype(np.float32).reshape(DH, 1)
    bvc = np.asarray(bv).astype(bf).astype(np.float32).reshape(DH, 1)

    tri = mask_f[:128, :128].astype(bf)       # causal triangle (0/-1e9)
    zeros = np.zeros((128, 128), dtype=bf)
    neg = np.full((128, 128), -1e9, dtype=np.float32).astype(bf)

    in_maps = []
    for c in range(N_CORES):
        b, h = divmod(c, 2)
        own = np.concatenate([np.arange(t * 128, (t + 1) * 128)
                              for t in range(h, 16, 2)])
        xcols = xb[b][own]                            # [1024, D]
        # [st, s, i, p] -> [st, p, i, s], flatten stage into rows
        xa = np.ascontiguousarray(
            xcols.reshape(2, ST, D_CH, 128).transpose(0, 3, 2, 1)
        ).reshape(2 * 128, D_CH, ST)
        if h == 0:
            msk = np.concatenate([tri, neg], axis=1)       # rank0 diag, rank1 future
        else:
            msk = np.concatenate([zeros, tri], axis=1)     # rank0 past, rank1 diag
        in_maps.append({
            "xT": xa, "mask": np.ascontiguousarray(msk),
            "wqT": wqt, "wkT": wkt, "wvT": wvt,
            "bq": bqc, "bk": bkc, "bv": bvc,
        })
    return in_maps


_NC_CACHE = {}


def kernel(x, attn_mask, Wq, bq, Wk, bk, Wv, bv):
    if "nc" not in _NC_CACHE:
        _NC_CACHE["nc"] = build_nc()
    nc = _NC_CACHE["nc"]
    in_maps = shard_inputs(x, attn_mask, Wq, bq, Wk, bk, Wv, bv)
    res = run_bass_kernel_spmd(nc, in_maps, list(range(N_CORES)))
    out = np.empty((B, S, DH), dtype=ml_dtypes.bfloat16)
    for c in range(N_CORES):
        b, h = divmod(c, 2)
        for j in range(NT):
            t = 2 * j + h
            out[b, t * 128:(t + 1) * 128, :] = res.results[c]["out"][j * 128:(j + 1) * 128]
    return out


# revision 33
# speedup vs baseline: 1.0917x; 1.0150x over previous
"""Trainium2 Bass kernel for a single attention head (B=4, S=2048, D=4096, DH=128).

Sharding: 8 cores = (batch b, half h), pair (2b, 2b+1) shares batch b.
Core (b,h) owns the INTERLEAVED tile set {t : t % 2 == h} (8 tiles of 128
positions) -- both its q rows AND its K/V key chunks. Each core projects
Q, K, V only for its OWN 1024 columns of x (half the work of replicating
K/V), then the pair exchanges K/V halves with pairwise AllGathers.

Rank-symmetric layout (same NEFF on every core): K^T / V go through a
DRAM bounce + AllGather and are read back RANK-indexed (rank0 = even
tiles, rank1 = odd tiles), so no access pattern depends on h. The causal
structure relative to rank0/rank1 is baked into the host-provided mask
block (triangle / zeros / -1e9).

Phases:
  1: two column-stages (own cols 0:512, 512:1024), K/V projections only
     (Q deferred). Gather 1 (K stage A) fires as early as possible so
     small tiles' logits can interleave with the deferred Q matmuls;
     gather 2 (K stage B + V both stages) fires after stage B drains and
     its latency hides under Q + small-tile work.
  2: per local q-tile j (global 2j+h): logits over rank0 range
     [0:(j+1)*128] and rank1 range [1024:1024+(j+1)*128], mask on the
     last chunk of each range; per-group rowmax (bf16 rounding is
     monotone so group-max == full max) -> exp (accum rowsum) -> W^T via
     one DMA transpose -> PV -> scale -> out. ALL logits precede ALL pv
     stages (the PE queue is in-order; a stalled pv would head-of-line
     block independent logits). Output DMAs ride gpsimd, transposes
     sync, exp scalar, everything else vector.
"""

import numpy as np
import ml_dtypes

import concourse.bass as bass
import concourse.tile as tile
from concourse import bacc, mybir
from concourse.bass_utils import run_bass_kernel_spmd

B, S, D, DH = 4, 2048, 4096, 128
SQ = S // 2          # own q rows / own key cols per core
ST = 512             # columns per stage
N_CORES = 8
D_CH = D // 128      # 32 contraction chunks
NT = 8               # local q tiles (slots)
PAIRS = [[0, 1], [2, 3], [4, 5], [6, 7]]

BF16 = mybir.dt.bfloat16
F32 = mybir.dt.float32


def build_nc():
    nc = bacc.Bacc(None)

    # x own columns, stage-major: xT[st*128+p, i, s] = x[b, col(st*512+s), i*128+p]
    xT = nc.dram_tensor("xT", [2 * 128, D_CH, ST], BF16, kind="ExternalInput")
    mask = nc.dram_tensor("mask", [128, 256], BF16, kind="ExternalInput")
    # weights pre-tiled on host: w[p, i, m] = W[m, i*128+p]
    wqT = nc.dram_tensor("wqT", [128, D_CH, DH], BF16, kind="ExternalInput")
    wkT = nc.dram_tensor("wkT", [128, D_CH, DH], BF16, kind="ExternalInput")
    wvT = nc.dram_tensor("wvT", [128, D_CH, DH], BF16, kind="ExternalInput")
    bq = nc.dram_tensor("bq", [DH, 1], F32, kind="ExternalInput")
    bk = nc.dram_tensor("bk", [DH, 1], F32, kind="ExternalInput")
    bv = nc.dram_tensor("bv", [DH, 1], F32, kind="ExternalInput")
    out = nc.dram_tensor("out", [SQ, DH], BF16, kind="ExternalOutput")
    # comm bounces: gather 1 = K^T stage A (early -- unlocks small tiles'
    # logits during the deferred-Q window); gather 2 = K^T stage B + V both
    cck_in = nc.dram_tensor("cck_in", [128, ST], BF16, kind="Internal")
    cck_out = nc.dram_tensor("cck_out", [256, ST], BF16, kind="Internal")
    cc2_in = nc.dram_tensor("cc2_in", [128, 3 * ST], BF16, kind="Internal")
    cc2_out = nc.dram_tensor("cc2_out", [256, 3 * ST], BF16, kind="Internal")

    with tile.TileContext(nc) as tc:
        with (
            tc.tile_pool(name="weights", bufs=1) as wpool,
            tc.tile_pool(name="persist", bufs=1) as persist,
        ):
            w_sb = {}
            for name in ("q", "k", "v"):
                w_sb[name] = wpool.tile([128, D_CH, DH], BF16, tag=f"w{name}",
                                        name=f"w{name}")
            W_EXT = {"q": wqT, "k": wkT, "v": wvT}
            b_sb = {}
            for name in ("q", "k", "v"):
                t = wpool.tile([DH, 1], F32, tag=f"b{name}", name=f"b{name}")
                b_sb[name] = t
            mk = persist.tile([128, 256], BF16, tag="mk")
            nc.gpsimd.dma_start(out=mk[:], in_=mask[:])
            for name, ext in (("q", bq), ("k", bk), ("v", bv)):
                nc.gpsimd.dma_start(out=b_sb[name][:], in_=ext[:])

            # rank-indexed: rank0 keys at [0:1024], rank1 at [1024:2048]
            kt_sb = persist.tile([DH, S], BF16, tag="kt")
            v_sb = persist.tile([128, 2 * NT, DH], BF16, tag="v")
            qt_sb = persist.tile([DH, SQ], BF16, tag="qt")   # own Q^T
            x_st = [persist.tile([128, D_CH, ST], BF16, tag=f"x{st}",
                                 name=f"x{st}") for st in (0, 1)]
            kst = [persist.tile([DH, ST], BF16, tag=f"kst{st}", name=f"kst{st}")
                   for st in (0, 1)]
            vst = [persist.tile([DH, ST], BF16, tag=f"vst{st}", name=f"vst{st}")
                   for st in (0, 1)]
            vtr = [persist.tile([128, 4, DH], BF16, tag=f"vtr{st}", name=f"vtr{st}")
                   for st in (0, 1)]

            # Q accumulators persist into phase 2 (small-tile logits
            # interleave between Q_A and Q_B): own 2 PSUM banks for the
            # whole kernel; K/V accumulators get a pool closed before
            # phase 2 opens its logits pool. 2 + 4 (kv) <= 8; later
            # 2 + 4 (lg) + 2 (o) = 8.
            ppq_cm = tc.tile_pool(name="ppsumq", bufs=1, space="PSUM")
            ppq = ppq_cm.__enter__()
            acc = {}
            for tag in ("pq0", "pq1"):
                acc[tag] = ppq.tile([DH, ST], F32, tag=tag, name=tag)

            # --- phase 1: K/V projections in two column-stages + gathers ---
            with tc.tile_pool(name="ppsumkv", bufs=1, space="PSUM") as ppkv:
                for tag in ("pk0", "pv0", "pk1", "pv1"):
                    acc[tag] = ppkv.tile([DH, ST], F32, tag=tag, name=tag)

                def x_piece(st, i0, n):
                    nc.sync.dma_start(out=x_st[st][:, i0:i0 + n, :],
                                      in_=xT[st * 128:(st + 1) * 128, i0:i0 + n, :])

                def wsl(name, i0, n):
                    ss = np.s_[:, i0:i0 + n, :]
                    nc.sync.dma_start(out=w_sb[name][ss], in_=W_EXT[name][ss])

                # single-queue x/w stream in consumption order (k/v weights
                # early, q weights before stage-B x since Q is deferred);
                # nothing else rides sync in phase 1, so the stream is
                # never blocked behind a wait
                x_piece(0, 0, 2)
                wsl("k", 0, 4)
                wsl("v", 0, 4)
                x_piece(0, 2, 2)
                x_piece(0, 4, 4)
                wsl("k", 4, 12)
                wsl("v", 4, 12)
                x_piece(0, 8, 8)
                x_piece(0, 16, 8)
                wsl("k", 16, 16)
                wsl("v", 16, 16)
                x_piece(0, 24, 8)
                wsl("q", 0, 16)
                wsl("q", 16, 16)
                for i0 in (0, 8, 16, 24):
                    x_piece(1, i0, 8)

                def stage_mms(st):
                    for i in range(D_CH):
                        stt = dict(start=(i == 0), stop=(i == D_CH - 1))
                        nc.tensor.matmul(acc[f"pk{st}"][:], lhsT=w_sb["k"][:, i, :],
                                         rhs=x_st[st][:, i, :], **stt)
                        nc.tensor.matmul(acc[f"pv{st}"][:], lhsT=w_sb["v"][:, i, :],
                                         rhs=x_st[st][:, i, :], **stt)

                def stage_drain(st):
                    nc.vector.tensor_scalar_add(kst[st][:], acc[f"pk{st}"][:],
                                                b_sb["k"][:])
                    nc.vector.tensor_scalar_add(vst[st][:], acc[f"pv{st}"][:],
                                                b_sb["v"][:])
                    nc.scalar.dma_start_transpose(out=vtr[st][:], in_=vst[st][:])

                stage_mms(0)
                stage_drain(0)
                stage_mms(1)
                stage_drain(1)
                # gather 1: K stage A (bounce/readback DMAs ride the
                # scalar HWDGE queue -- gpsimd DMA is the slow SWDGE path)
                nc.scalar.dma_start(out=cck_in.ap(), in_=kst[0][:])
                nc.gpsimd.collective_compute(
                    "AllGather", mybir.AluOpType.bypass,
                    replica_groups=PAIRS,
                    ins=[cck_in.ap().opt()], outs=[cck_out.ap().opt()],
                )
                # gather 2 bounces queue before gather 1's readbacks so
                # they are not blocked behind the ccA-completion wait
                nc.scalar.dma_start(out=cc2_in[:, 0:ST], in_=kst[1][:])
                nc.scalar.dma_start(out=cc2_in[:, ST:2 * ST], in_=vtr[0][:])
                nc.scalar.dma_start(out=cc2_in[:, 2 * ST:3 * ST], in_=vtr[1][:])
                nc.gpsimd.collective_compute(
                    "AllGather", mybir.AluOpType.bypass,
                    replica_groups=PAIRS,
                    ins=[cc2_in.ap().opt()], outs=[cc2_out.ap().opt()],
                )
                nc.scalar.dma_start(out=kt_sb[:, 0:ST], in_=cck_out[0:128, :])
                nc.scalar.dma_start(out=kt_sb[:, SQ:SQ + ST],
                                    in_=cck_out[128:256, :])
                nc.scalar.dma_start(out=kt_sb[:, ST:SQ], in_=cc2_out[0:128, 0:ST])
                nc.scalar.dma_start(out=kt_sb[:, SQ + ST:S],
                                    in_=cc2_out[128:256, 0:ST])
                nc.scalar.dma_start(out=v_sb[:, 0:NT, :],
                                    in_=cc2_out[0:128, ST:3 * ST])
                nc.scalar.dma_start(out=v_sb[:, NT:2 * NT, :],
                                    in_=cc2_out[128:256, ST:3 * ST])

            # --- phase 2 (deferred Q interleaved) ---
            with (
                tc.tile_pool(name="lg_psum", bufs=4, space="PSUM") as lg,
                tc.tile_pool(name="o_psum", bufs=2, space="PSUM") as opool,
                tc.tile_pool(name="lm_sb", bufs=1) as lmpool,
                tc.tile_pool(name="wt_sb", bufs=1) as wtpool,
                tc.tile_pool(name="stats", bufs=12) as stat,
                tc.tile_pool(name="out_sb", bufs=2) as ospool,
            ):
                pv_args = {}

                def q_proj(st):
                    for i in range(D_CH):
                        nc.tensor.matmul(acc[f"pq{st}"][:], lhsT=w_sb["q"][:, i, :],
                                         rhs=x_st[st][:, i, :],
                                         start=(i == 0), stop=(i == D_CH - 1))
                    nc.vector.tensor_scalar_add(qt_sb[:, st * ST:(st + 1) * ST],
                                                acc[f"pq{st}"][:], b_sb["q"][:])

                def softmax_stage(j):
                    e = j + 1            # chunks per rank range
                    w = e * 128          # cols per rank range
                    qsl = np.s_[:, j * 128:(j + 1) * 128]

                    # Only the LAST chunk of each rank range carries mask
                    # values (host bakes h into mk): rank0 last = triangle
                    # (h=0) or all-0 (h=1); rank1 last = all--1e9 (h=0) or
                    # triangle (h=1). Other chunks are pure past: plain
                    # psum f32 -> bf16 rounding. Rowmax is per-group as
                    # each group's bf16 logits land (rounding is monotone,
                    # so group-wise max == full max) + one tiny combine.
                    lmt = lmpool.tile([128, 2 * w], BF16, tag=f"lm{j}")
                    gms = stat.tile([128, 8], F32, tag="gms")
                    ng = 0
                    for base, off, mcol in ((0, 0, 0), (SQ, w, 128)):
                        for g0 in range(0, w, 512):
                            gw = min(512, w - g0)
                            pg = lg.tile([128, 512], F32, tag="pg")
                            nc.tensor.matmul(pg[:, :gw], lhsT=qt_sb[qsl],
                                             rhs=kt_sb[:, base + g0:base + g0 + gw],
                                             start=True, stop=True)
                            last = g0 + gw == w
                            cp = gw - 128 if last else gw
                            if cp:
                                nc.vector.tensor_copy(
                                    lmt[:, off + g0:off + g0 + cp], pg[:, :cp])
                            if last:
                                nc.vector.tensor_add(
                                    lmt[:, off + g0 + cp:off + g0 + gw],
                                    pg[:, cp:gw],
                                    mk[:, mcol:mcol + 128])
                            nc.vector.reduce_max(
                                out=gms[:, ng:ng + 1],
                                in_=lmt[:, off + g0:off + g0 + gw],
                                axis=mybir.AxisListType.X)
                            ng += 1
                    negmax = stat.tile([128, 1], F32, tag="negmax")
                    nc.vector.reduce_max(out=negmax[:], in_=gms[:, :ng],
                                         axis=mybir.AxisListType.X, negate=True)
                    w_t = lmpool.tile([128, 2 * w], BF16, tag=f"w{j}")
                    wt_t = wtpool.tile([128, 2 * e, 128], BF16, tag=f"wt{j}")
                    sumexp = stat.tile([128, 1], F32, tag="sumexp")
                    nc.scalar.activation(
                        out=w_t[:, :2 * w], in_=lmt[:, :2 * w],
                        func=mybir.ActivationFunctionType.Exp,
                        bias=negmax[:], scale=1.0, accum_out=sumexp[:])
                    nc.sync.dma_start_transpose(out=wt_t[:, :2 * e, :],
                                                in_=w_t[:, :2 * w])
                    pv_args[j] = (wt_t, sumexp, e)

                def pv_stage(j):
                    wt_t, sumexp, e = pv_args.pop(j)
                    rsum = stat.tile([128, 1], F32, tag="rsum")
                    nc.vector.reciprocal(rsum[:], sumexp[:])
                    po = opool.tile([128, DH], F32, tag="po")
                    for c in range(2 * e):
                        vc = c if c < e else NT + (c - e)
                        nc.tensor.matmul(po[:], lhsT=wt_t[:, c, :], rhs=v_sb[:, vc, :],
                                         start=(c == 0), stop=(c == 2 * e - 1))
                    o_sb = ospool.tile([128, DH], BF16, tag="o")
                    nc.vector.tensor_scalar_mul(o_sb[:], po[:], rsum[:])
                    nc.scalar.dma_start(out=out[j * 128:(j + 1) * 128, :],
                                        in_=o_sb[:])

                # PE program order: Q_A -> small-tile logits (gated only on
                # gather 1) -> Q_B -> big-tile logits (gather 2 arrives
                # under Q/smalls) -> ALL pv stages (in chain-completion
                # order). ALL logits precede ALL pv: the PE queue is
                # in-order and a stalled pv would block independent logits.
                q_proj(0)
                softmax_stage(3)
                softmax_stage(2)
                softmax_stage(1)
                softmax_stage(0)
                q_proj(1)
                softmax_stage(7)
                softmax_stage(6)
                softmax_stage(5)
                softmax_stage(4)
                pv_stage(3)
                pv_stage(2)
                pv_stage(1)
                pv_stage(0)
                pv_stage(7)
                pv_stage(6)
                pv_stage(5)
                pv_stage(4)
            ppq_cm.__exit__(None, None, None)

    nc.finalize()
    return nc


def shard_inputs(x, attn_mask, Wq, bq, Wk, bk, Wv, bv):
    """Host-side shard prep. Returns in_maps for cores 0..7."""
    bf = ml_dtypes.bfloat16
    xb = np.asarray(x).astype(bf)                   # cast first, like the reference
    mask_f = np.asarray(attn_mask)

    def tile_w(W):
        WT = np.asarray(W).astype(bf).T.reshape(D_CH, 128, DH)
        return np.ascontiguousarray(WT.transpose(1, 0, 2))

    wqt, wkt, wvt = tile_w(Wq), tile_w(Wk), tile_w(Wv)
    bqc = np.asarray(bq).astype(bf).astype(np.float32).reshape(DH, 1)
    bkc = np.asarray(bk).astype(bf).astype(np.float32).reshape(DH, 1)
    bvc = np.asarray(bv).astype(bf).astype(np.float32).reshape(DH, 1)

    tri = mask_f[:128, :128].astype(bf)       # causal triangle (0/-1e9)
    zeros = np.zeros((128, 128), dtype=bf)
    neg = np.full((128, 128), -1e9, dtype=np.float32).astype(bf)

    in_maps = []
    for c in range(N_CORES):
        b, h = divmod(c, 2)
        own = np.concatenate([np.arange(t * 128, (t + 1) * 128)
                              for t in range(h, 16, 2)])
        xcols = xb[b][own]                            # [1024, D]
        # [st, s, i, p] -> [st, p, i, s], flatten stage into rows
        xa = np.ascontiguousarray(
            xcols.reshape(2, ST, D_CH, 128).transpose(0, 3, 2, 1)
        ).reshape(2 * 128, D_CH, ST)
        if h == 0:
            msk = np.concatenate([tri, neg], axis=1)       # rank0 diag, rank1 future
        else:
            msk = np.concatenate([zeros, tri], axis=1)     # rank0 past, rank1 diag
        in_maps.append({
            "xT": xa, "mask": np.ascontiguousarray(msk),
            "wqT": wqt, "wkT": wkt, "wvT": wvt,
            "bq": bqc, "bk": bkc, "bv": bvc,
        })
    return in_maps


_NC_CACHE = {}


def kernel(x, attn_mask, Wq, bq, Wk, bk, Wv, bv):
    if "nc" not in _NC_CACHE:
        _NC_CACHE["nc"] = build_nc()
    nc = _NC_CACHE["nc"]
    in_maps = shard_inputs(x, attn_mask, Wq, bq, Wk, bk, Wv, bv)
    res = run_bass_kernel_spmd(nc, in_maps, list(range(N_CORES)))
    out = np.empty((B, S, DH), dtype=ml_dtypes.bfloat16)
    for c in range(N_CORES):
        b, h = divmod(c, 2)
        for j in range(NT):
            t = 2 * j + h
            out[b, t * 128:(t + 1) * 128, :] = res.results[c]["out"][j * 128:(j + 1) * 128]
    return out
